# revision 1
# baseline (speedup 1.0000x reference)
"""Trainium2 Bass kernel v2: PINN MLP forward + JVP derivs (T, Tz, Tt, Tzz).

Math per point (feature-major), layer l: h = tanh(a), g = 1 - h^2:
  forward:      a_{l+1} = W^T h_l
  1st deriv:    a'_{l+1} = W^T (g_l * a'_l)         (z and t directions)
  2nd deriv(z): a''_{l+1} = W^T (g_l * (a''_l - 2 h_l a'_l^2))
L1 derivative seeds fold into host-precomputed W2z/W2t/W2zz.

Perf structure vs v1:
  - All L2+ matmuls in fp16 (1 cyc/row on PE vs 4 for fp32); L1 in fp32r.
  - z-chain carries sqrt(2) (folded into W2z and the W4 output column) so
    the 2*az^2 term is a plain square -- no scale op.
  - g is never materialized: ops use (h^2 - 1) via scalar_tensor_tensor,
    with the sign folded into -W3 / -W4 weight copies.
  - (az|at) PSUM pairs are converted to fp16 SBUF by one ACT copy, making
    downstream DVE ops 2-byte (2x mode) and GPSIMD-eligible (no PSUM port).
  - i = app - v runs on the PE as an accumulating (-I) matmul.
  - Layer 3 (width 64) processes TILE PAIRS packed on partitions; layer 4
    is 4 accumulating [128,8] matmuls producing all 8 output rows.
  - Elementwise is load-balanced: ACT (tanh+converts), DVE (squares,
    products), GPSIMD (stt forms), PE (adds).

Sharding: pure data parallel, 262144 points -> 8 cores x 32768.
"""

import sys

import numpy as np

sys.path.insert(0, "/opt/trn_rl_repo")

import concourse.bass as bass
import concourse.bacc as bacc
import concourse.tile as tile
from concourse import mybir
from concourse.bass_utils import run_bass_kernel_spmd

N = 262144
NCORES = 8
NSH = N // NCORES  # 32768 points per core
B = 512  # points per tile
NT = NSH // B  # 64 tiles
NP = NT // 2  # 32 tile pairs
CH = 4096  # x chunk (points) per input DMA
TPC = CH // B  # tiles per chunk

F32 = mybir.dt.float32
F32R = mybir.dt.float32r
F16 = mybir.dt.float16

TRACE = False
LAST_RESULT = None


def _r(ap):
    return ap.bitcast(F32R)


def _build():
    nc = bacc.Bacc(None, target_bir_lowering=False)

    xT = nc.declare_dram_parameter("xT", [3, NSH], F32R, isOutput=False)
    W1 = nc.declare_dram_parameter("W1", [3, 128], F32R, isOutput=False)
    W2 = nc.declare_dram_parameter("W2", [128, 128], F16, isOutput=False)
    W2z = nc.declare_dram_parameter("W2z", [128, 128], F16, isOutput=False)
    W2t = nc.declare_dram_parameter("W2t", [128, 128], F16, isOutput=False)
    W2zz = nc.declare_dram_parameter("W2zz", [128, 128], F16, isOutput=False)
    W3 = nc.declare_dram_parameter("W3", [128, 64], F16, isOutput=False)
    W3n = nc.declare_dram_parameter("W3n", [128, 64], F16, isOutput=False)
    W4h = nc.declare_dram_parameter("W4h", [128, 8], F16, isOutput=False)
    W4z = nc.declare_dram_parameter("W4z", [128, 8], F16, isOutput=False)
    W4t = nc.declare_dram_parameter("W4t", [128, 8], F16, isOutput=False)
    W4p = nc.declare_dram_parameter("W4p", [128, 8], F16, isOutput=False)
    negI = nc.declare_dram_parameter("negI", [128, 128], F16, isOutput=False)
    b1 = nc.declare_dram_parameter("b1", [128, 1], F32, isOutput=False)
    b2 = nc.declare_dram_parameter("b2", [128, 1], F32, isOutput=False)
    b3d = nc.declare_dram_parameter("b3d", [128, 1], F32, isOutput=False)
    out_d = nc.declare_dram_parameter("out", [4, NSH], F32, isOutput=True)

    Tanh = mybir.ActivationFunctionType.Tanh
    mult = mybir.AluOpType.mult
    sub = mybir.AluOpType.subtract
    add = mybir.AluOpType.add

    with tile.TileContext(nc) as tc:
        with (
            tc.tile_pool(name="consts", bufs=1) as consts,
            tc.tile_pool(name="xin", bufs=2) as xin,
            tc.tile_pool(name="work", bufs=4) as work,
            tc.tile_pool(name="psA", bufs=2, space="PSUM") as psA,
            tc.tile_pool(name="psZT", bufs=2, space="PSUM") as psZT,
            tc.tile_pool(name="psPP", bufs=2, space="PSUM") as psPP,
        ):
            W1s = consts.tile([3, 128], F32R)
            nc.sync.dma_start(out=W1s, in_=W1[:])
            W2s = consts.tile([128, 128], F16)
            nc.sync.dma_start(out=W2s, in_=W2[:])
            W2zs = consts.tile([128, 128], F16)
            nc.sync.dma_start(out=W2zs, in_=W2z[:])
            W2ts = consts.tile([128, 128], F16)
            nc.sync.dma_start(out=W2ts, in_=W2t[:])
            W2zzs = consts.tile([128, 128], F16)
            nc.sync.dma_start(out=W2zzs, in_=W2zz[:])
            W3s = consts.tile([128, 64], F16)
            nc.sync.dma_start(out=W3s, in_=W3[:])
            W3ns = consts.tile([128, 64], F16)
            nc.sync.dma_start(out=W3ns, in_=W3n[:])
            W4hs = consts.tile([128, 8], F16)
            nc.sync.dma_start(out=W4hs, in_=W4h[:])
            W4zs = consts.tile([128, 8], F16)
            nc.sync.dma_start(out=W4zs, in_=W4z[:])
            W4ts = consts.tile([128, 8], F16)
            nc.sync.dma_start(out=W4ts, in_=W4t[:])
            W4ps = consts.tile([128, 8], F16)
            nc.sync.dma_start(out=W4ps, in_=W4p[:])
            negIs = consts.tile([128, 128], F16)
            nc.sync.dma_start(out=negIs, in_=negI[:])
            b1s = consts.tile([128, 1], F32)
            nc.sync.dma_start(out=b1s, in_=b1[:])
            b2s = consts.tile([128, 1], F32)
            nc.sync.dma_start(out=b2s, in_=b2[:])
            b3ds = consts.tile([128, 1], F32)
            nc.sync.dma_start(out=b3ds, in_=b3d[:])

            xc = None

            def half_tile(t):
                """L1+L2 for one tile; returns (h2, mtz2, mtt2, mhpp2)."""
                nonlocal xc
                ci = t % TPC
                if ci == 0:
                    c0 = (t // TPC) * CH
                    xc = xin.tile([3, CH], F32R, tag="xc", name="xc")
                    nc.sync.dma_start(out=xc, in_=xT[:, c0 : c0 + CH])
                xs = xc[:, ci * B : (ci + 1) * B]

                # ---- layer 1 ----
                pa1 = psA.tile([128, B], F32, tag="pa", name="pa1")
                nc.tensor.matmul(pa1, W1s, xs)
                h1 = work.tile([128, B], F16, tag="h1", name="h1")
                nc.scalar.activation(out=h1, in_=pa1, func=Tanh, bias=b1s)
                hh1 = work.tile([128, B], F16, tag="hh1", name="hh1")
                nc.vector.tensor_mul(out=hh1, in0=h1, in1=h1)
                g1 = work.tile([128, B], F16, tag="g1", name="g1")
                nc.vector.tensor_scalar(
                    out=g1, in0=hh1, scalar1=-1.0, scalar2=1.0, op0=mult, op1=add
                )
                u1 = work.tile([128, B], F16, tag="u1", name="u1")
                nc.vector.tensor_mul(out=u1, in0=h1, in1=g1)

                # ---- layer 2 matmuls ----
                pa2 = psA.tile([128, B], F32, tag="pa", name="pa2")
                nc.tensor.matmul(pa2, W2s, h1)
                dzt2 = psZT.tile([128, 2 * B], F32, tag="zt", name="dzt2")
                nc.tensor.matmul(dzt2[:, 0:B], W2zs, g1)  # az' = sqrt2*az
                nc.tensor.matmul(dzt2[:, B : 2 * B], W2ts, g1)  # at
                papp2 = psPP.tile([128, B], F32, tag="pp", name="papp2")
                nc.tensor.matmul(papp2, W2zzs, u1, start=True, stop=False)

                # ---- layer 2 pointwise ----
                h2 = work.tile([128, B], F16, tag="h2", name="h2")
                nc.scalar.activation(out=h2, in_=pa2, func=Tanh, bias=b2s)
                zt2 = work.tile([128, 2 * B], F16, tag="zt2", name="zt2")
                nc.scalar.copy(out=zt2, in_=dzt2)  # az'|at -> fp16 SBUF
                az2 = zt2[:, 0:B]
                at2 = zt2[:, B : 2 * B]
                mg2 = work.tile([128, B], F16, tag="mg2", name="mg2")
                nc.vector.tensor_mul(out=mg2, in0=h2, in1=h2)
                # mg2 = h2^2 - 1 = -g2
                nc.vector.tensor_scalar(
                    out=mg2, in0=mg2, scalar1=1.0, scalar2=None, op0=sub
                )
                # -t2z' = mg2 * az', -t2t = mg2 * at  (GPSIMD)
                mtz2 = work.tile([128, B], F16, tag="mtz2", name="mtz2")
                nc.gpsimd.tensor_mul(out=mtz2, in0=mg2, in1=az2)
                mtt2 = work.tile([128, B], F16, tag="mtt2", name="mtt2")
                nc.gpsimd.tensor_mul(out=mtt2, in0=mg2, in1=at2)
                sq2 = work.tile([128, B], F16, tag="sq2", name="sq2")
                nc.vector.tensor_mul(out=sq2, in0=az2, in1=az2)  # 2*az^2
                v2 = work.tile([128, B], F16, tag="v2", name="v2")
                nc.vector.tensor_mul(out=v2, in0=h2, in1=sq2)
                # papp2 <- a2'' - v2  (PE accumulate)
                nc.tensor.matmul(papp2, negIs, v2, start=False, stop=True)
                return h2, mtz2, mtt2, mg2, papp2

            for p in range(NP):
                t0 = 2 * p
                ha, tza, tta, mga, p2a = half_tile(t0)
                hb, tzb, ttb, mgb, p2b = half_tile(t0 + 1)
                # -h2'' = mg2 * i2; deferred past the next half tile so the
                # DVE never head-of-line blocks on the PE's (-I) accumulate.
                ppa = work.tile([128, B], F16, tag="mhpp2", name="mhpp2a")
                nc.vector.tensor_mul(out=ppa, in0=mga, in1=p2a)

                # ---- layer 3 matmuls (pair-packed on partitions) ----
                pa3 = psA.tile([128, B], F32, tag="pa", name="pa3")
                nc.tensor.matmul(pa3[0:64], W3s, ha)
                nc.tensor.matmul(pa3[64:128], W3s, hb)
                dzt3 = psZT.tile([128, 2 * B], F32, tag="zt", name="dzt3")
                nc.tensor.matmul(dzt3[0:64, 0:B], W3ns, tza)  # a3z' tile a
                nc.tensor.matmul(dzt3[64:128, 0:B], W3ns, tzb)
                nc.tensor.matmul(dzt3[0:64, B : 2 * B], W3ns, tta)  # a3t
                nc.tensor.matmul(dzt3[64:128, B : 2 * B], W3ns, ttb)
                # mhpp2b deferred past the pa3/dzt3 matmuls: its (-I)
                # accumulate finishes last, so it gets the extra PE slack.
                ppb = work.tile([128, B], F16, tag="mhpp2", name="mhpp2b")
                nc.vector.tensor_mul(out=ppb, in0=mgb, in1=p2b)
                papp3 = psPP.tile([128, B], F32, tag="pp", name="papp3")
                nc.tensor.matmul(papp3[0:64], W3ns, ppa, start=True, stop=False)
                nc.tensor.matmul(papp3[64:128], W3ns, ppb, start=True, stop=False)

                # ---- layer 3 pointwise ----
                h3 = work.tile([128, B], F16, tag="h3", name="h3")
                nc.scalar.activation(out=h3, in_=pa3, func=Tanh, bias=b3ds)
                zt3 = work.tile([128, 2 * B], F16, tag="zt3", name="zt3")
                nc.scalar.copy(out=zt3, in_=dzt3)
                az3 = zt3[:, 0:B]
                at3 = zt3[:, B : 2 * B]
                mg3 = work.tile([128, B], F16, tag="mg3", name="mg3")
                nc.vector.tensor_mul(out=mg3, in0=h3, in1=h3)
                nc.vector.tensor_scalar(
                    out=mg3, in0=mg3, scalar1=1.0, scalar2=None, op0=sub
                )
                mtz3 = work.tile([128, B], F16, tag="mtz3", name="mtz3")
                nc.gpsimd.tensor_mul(out=mtz3, in0=mg3, in1=az3)
                mtt3 = work.tile([128, B], F16, tag="mtt3", name="mtt3")
                nc.gpsimd.tensor_mul(out=mtt3, in0=mg3, in1=at3)
                sq3 = work.tile([128, B], F16, tag="sq3", name="sq3")
                nc.vector.tensor_mul(out=sq3, in0=az3, in1=az3)
                v3 = work.tile([128, B], F16, tag="v3", name="v3")
                nc.vector.tensor_mul(out=v3, in0=h3, in1=sq3)
                nc.tensor.matmul(papp3, negIs, v3, start=False, stop=True)
                mhpp3 = work.tile([128, B], F16, tag="mhpp3", name="mhpp3")
                nc.vector.tensor_mul(out=mhpp3, in0=mg3, in1=papp3)

                # ---- layer 4: accumulate all 8 output rows ----
                p4 = psPP.tile([8, B], F32, tag="pp", name="p4")
                nc.tensor.matmul(p4, W4hs, h3, start=True, stop=False)
                nc.tensor.matmul(p4, W4zs, mtz3, start=False, stop=False)
                nc.tensor.matmul(p4, W4ts, mtt3, start=False, stop=False)
                nc.tensor.matmul(p4, W4ps, mhpp3, start=False, stop=True)
                sb4 = work.tile([8, B], F32, tag="sb4", name="sb4")
                nc.scalar.copy(out=sb4, in_=p4)
                ofull = out_d[:]
                o8 = bass.AP(
                    tensor=ofull.tensor,
                    offset=ofull.offset + t0 * B,
                    ap=[[B, 2], [NSH, 4], [1, B]],
                )
                nc.sync.dma_start(out=o8, in_=sb4)

    nc.finalize()
    return nc


_NC_CACHE = None


def _get_nc():
    global _NC_CACHE
    if _NC_CACHE is None:
        _NC_CACHE = _build()
    return _NC_CACHE


def kernel(**inputs):
    global LAST_RESULT
    f = np.float32
    f16 = np.float16
    x = np.asarray(inputs["x"], dtype=f)
    W1 = np.asarray(inputs["W1"], dtype=f)
    b1 = np.asarray(inputs["b1"], dtype=f)
    W2 = np.asarray(inputs["W2"], dtype=f)
    b2 = np.asarray(inputs["b2"], dtype=f)
    W3 = np.asarray(inputs["W3"], dtype=f)
    b3 = np.asarray(inputs["b3"], dtype=f)
    W4 = np.asarray(inputs["W4"], dtype=f)
    b4 = np.asarray(inputs["b4"], dtype=f)

    xT = np.ascontiguousarray(x.T)  # [3, N]
    w4 = W4[:, 0].astype(f)
    SQ2 = np.sqrt(2.0).astype(f)

    W4h = np.zeros((128, 8), f)
    W4h[0:64, 0] = w4
    W4h[64:128, 4] = w4
    W4z = np.zeros((128, 8), f)
    W4z[0:64, 1] = -w4 / SQ2
    W4z[64:128, 5] = -w4 / SQ2
    W4t = np.zeros((128, 8), f)
    W4t[0:64, 2] = -w4
    W4t[64:128, 6] = -w4
    W4p = np.zeros((128, 8), f)
    W4p[0:64, 3] = -w4
    W4p[64:128, 7] = -w4

    common = {
        "W1": W1,
        "W2": W2.astype(f16),
        "W2z": (SQ2 * W1[0][:, None] * W2).astype(f16),
        "W2t": (W1[1][:, None] * W2).astype(f16),
        "W2zz": (-2.0 * (W1[0] ** 2)[:, None] * W2).astype(f16),
        "W3": W3.astype(f16),
        "W3n": (-W3).astype(f16),
        "W4h": W4h.astype(f16),
        "W4z": W4z.astype(f16),
        "W4t": W4t.astype(f16),
        "W4p": W4p.astype(f16),
        "negI": (-np.eye(128)).astype(f16),
        "b1": np.ascontiguousarray(b1.reshape(128, 1)),
        "b2": np.ascontiguousarray(b2.reshape(128, 1)),
        "b3d": np.ascontiguousarray(np.concatenate([b3, b3]).reshape(128, 1)),
    }
    in_maps = [
        dict(common, xT=np.ascontiguousarray(xT[:, i * NSH : (i + 1) * NSH]))
        for i in range(NCORES)
    ]

    nc = _get_nc()
    res = run_bass_kernel_spmd(nc, in_maps, list(range(NCORES)), trace=TRACE)
    LAST_RESULT = res

    full = np.concatenate(
        [res.results[i]["out"] for i in range(NCORES)], axis=1
    )  # [4, N] rows (T, Tz, Tt, Tpp)
    out = np.ascontiguousarray(full.T).astype(f)
    out[:, 0] += b4[0]
    return out



# revision 4
# speedup vs baseline: 1.4973x; 1.4973x over previous
"""Trainium2 Bass kernel v3: PINN MLP forward + JVP derivs (T, Tz, Tt, Tzz).

Math per point (feature-major), layer l: h = tanh(a), g = 1 - h^2:
  forward:      a_{l+1} = W^T h_l
  1st deriv:    a'_{l+1} = W^T (g_l * a'_l)         (z and t directions)
  2nd deriv(z): a''_{l+1} = W^T (g_l * (a''_l - 2 h_l a'_l^2))
L1 derivative seeds fold into host-precomputed Z2/T2/ZZ2.

v3 structure (vs v2): everything fp16 on the PE (incl. L1); g is never
materialized - all (g*x) products are scalar_tensor_tensor ops
(hh - 1) * x reading the derivative preactivations DIRECTLY from PSUM
(no ACT copies of az|at, no GPSIMD narrow muls of fp16 pairs); sq from
PSUM via ACT Square; elementwise runs at width >= 1024 (the fp16
TT/TS 512-wide DVE slow path is avoided); sign bookkeeping makes W3
shared by the forward/z/t chains at L3.

Engine split per pair (2 x 512 points):
  PE:     25 matmul streams (L1 2, L2 8, negI 3, L3 8, L4 4)
  ACT:    tanh x5, Square(az_psum) x3, p4 copy
  DVE:    mg1 TS, u1n/hh2/hh3 TT-1024, mzt STT-1024(PSUM), mhpp STT
  GPSIMD: hh1 x2, v2 x2, v3 (SBUF fp16 narrow muls)

Sharding: pure data parallel, 262144 points -> 8 cores x 32768.
"""

import sys

import numpy as np

sys.path.insert(0, "/opt/trn_rl_repo")

import concourse.bass as bass
import concourse.bacc as bacc
import concourse.tile as tile
from concourse import mybir
from concourse.bass_utils import run_bass_kernel_spmd

N = 262144
NCORES = 8
NSH = N // NCORES  # 32768 points per core
B = 512  # points per tile
NT = NSH // B  # 64 tiles
NP = NT // 2  # 32 tile pairs
CH = 4096  # x chunk (points) per input DMA
TPC = CH // B  # tiles per chunk

F32 = mybir.dt.float32
F16 = mybir.dt.float16

TRACE = False
LAST_RESULT = None


def _build():
    nc = bacc.Bacc(None, target_bir_lowering=False)

    xT = nc.declare_dram_parameter("xT", [3, NSH], F16, isOutput=False)
    W1 = nc.declare_dram_parameter("W1", [3, 128], F16, isOutput=False)
    W2 = nc.declare_dram_parameter("W2", [128, 128], F16, isOutput=False)
    Z2 = nc.declare_dram_parameter("Z2", [128, 128], F16, isOutput=False)
    T2 = nc.declare_dram_parameter("T2", [128, 128], F16, isOutput=False)
    ZZ2 = nc.declare_dram_parameter("ZZ2", [128, 128], F16, isOutput=False)
    W3 = nc.declare_dram_parameter("W3", [128, 64], F16, isOutput=False)
    W3n = nc.declare_dram_parameter("W3n", [128, 64], F16, isOutput=False)
    W4h = nc.declare_dram_parameter("W4h", [128, 8], F16, isOutput=False)
    W4z = nc.declare_dram_parameter("W4z", [128, 8], F16, isOutput=False)
    W4t = nc.declare_dram_parameter("W4t", [128, 8], F16, isOutput=False)
    W4p = nc.declare_dram_parameter("W4p", [128, 8], F16, isOutput=False)
    negI = nc.declare_dram_parameter("negI", [128, 128], F16, isOutput=False)
    b1 = nc.declare_dram_parameter("b1", [128, 1], F32, isOutput=False)
    b2 = nc.declare_dram_parameter("b2", [128, 1], F32, isOutput=False)
    b3d = nc.declare_dram_parameter("b3d", [128, 1], F32, isOutput=False)
    out_d = nc.declare_dram_parameter("out", [4, NSH], F32, isOutput=True)

    Tanh = mybir.ActivationFunctionType.Tanh
    Square = mybir.ActivationFunctionType.Square
    mult = mybir.AluOpType.mult
    sub = mybir.AluOpType.subtract

    def stt(eng, out, in0, in1):
        # out = (in0 - 1) * in1
        eng.scalar_tensor_tensor(
            out=out, in0=in0, scalar=1.0, in1=in1, op0=sub, op1=mult
        )

    def bc2(t_ap):
        """[128,512] AP -> broadcast [128, 2x512] (block repeated twice)."""
        return bass.AP(
            tensor=t_ap.tensor,
            offset=t_ap.offset,
            ap=[t_ap.ap[0], [0, 2], [1, B]],
        )

    def split2(t_ap, half_stride):
        """[128, 2*B] write AP whose halves land `half_stride` cols apart."""
        return bass.AP(
            tensor=t_ap.tensor,
            offset=t_ap.offset,
            ap=[t_ap.ap[0], [half_stride, 2], [1, B]],
        )

    with tile.TileContext(nc) as tc:
        with (
            tc.tile_pool(name="consts", bufs=1) as consts,
            tc.tile_pool(name="xin", bufs=2) as xin,
            tc.tile_pool(name="l1", bufs=2) as l1p,
            tc.tile_pool(name="l2", bufs=2) as l2p,
            tc.tile_pool(name="l3", bufs=2) as l3p,
            tc.tile_pool(name="sb4", bufs=2) as sb4p,
            tc.tile_pool(name="paP", bufs=2, space="PSUM") as paP,
            tc.tile_pool(name="dztP", bufs=2, space="PSUM") as dztP,
            tc.tile_pool(name="appP", bufs=2, space="PSUM") as appP,
        ):
            W1s = consts.tile([3, 128], F16)
            nc.sync.dma_start(out=W1s, in_=W1[:])
            W2s = consts.tile([128, 128], F16)
            nc.sync.dma_start(out=W2s, in_=W2[:])
            Z2s = consts.tile([128, 128], F16)
            nc.sync.dma_start(out=Z2s, in_=Z2[:])
            T2s = consts.tile([128, 128], F16)
            nc.sync.dma_start(out=T2s, in_=T2[:])
            ZZ2s = consts.tile([128, 128], F16)
            nc.sync.dma_start(out=ZZ2s, in_=ZZ2[:])
            W3s = consts.tile([128, 64], F16)
            nc.sync.dma_start(out=W3s, in_=W3[:])
            W3ns = consts.tile([128, 64], F16)
            nc.sync.dma_start(out=W3ns, in_=W3n[:])
            W4hs = consts.tile([128, 8], F16)
            nc.sync.dma_start(out=W4hs, in_=W4h[:])
            W4zs = consts.tile([128, 8], F16)
            nc.sync.dma_start(out=W4zs, in_=W4z[:])
            W4ts = consts.tile([128, 8], F16)
            nc.sync.dma_start(out=W4ts, in_=W4t[:])
            W4ps = consts.tile([128, 8], F16)
            nc.sync.dma_start(out=W4ps, in_=W4p[:])
            negIs = consts.tile([128, 128], F16)
            nc.sync.dma_start(out=negIs, in_=negI[:])
            b1s = consts.tile([128, 1], F32)
            nc.sync.dma_start(out=b1s, in_=b1[:])
            b2s = consts.tile([128, 1], F32)
            nc.sync.dma_start(out=b2s, in_=b2[:])
            b3ds = consts.tile([128, 1], F32)
            nc.sync.dma_start(out=b3ds, in_=b3d[:])

            xc = None

            for p in range(NP):
                ta, tb = 2 * p, 2 * p + 1
                ci = ta % TPC
                if ci == 0:
                    c0 = (ta // TPC) * CH
                    xc = xin.tile([3, CH], F16, tag="xc", name="xc")
                    nc.sync.dma_start(out=xc, in_=xT[:, c0 : c0 + CH])
                xa = xc[:, ci * B : (ci + 1) * B]
                xb = xc[:, (ci + 1) * B : (ci + 2) * B]

                # ================= layer 1 =================
                pa1a = paP.tile([128, B], F32, tag="pa", name="pa1a")
                nc.tensor.matmul(pa1a, W1s, xa)
                pa1b = paP.tile([128, B], F32, tag="pa", name="pa1b")
                nc.tensor.matmul(pa1b, W1s, xb)

                h1w = l1p.tile([128, 2 * B], F16, tag="h1w", name="h1w")
                nc.scalar.activation(out=h1w[:, 0:B], in_=pa1a, func=Tanh, bias=b1s)
                nc.scalar.activation(
                    out=h1w[:, B : 2 * B], in_=pa1b, func=Tanh, bias=b1s
                )
                hh1w = l1p.tile([128, 2 * B], F16, tag="hh1w", name="hh1w")
                nc.gpsimd.tensor_mul(
                    out=hh1w[:, 0:B], in0=h1w[:, 0:B], in1=h1w[:, 0:B]
                )
                nc.gpsimd.tensor_mul(
                    out=hh1w[:, B : 2 * B], in0=h1w[:, B : 2 * B],
                    in1=h1w[:, B : 2 * B],
                )
                # mg1 = hh1 - 1 = -g1  (TS-1024)
                mg1w = l1p.tile([128, 2 * B], F16, tag="mg1w", name="mg1w")
                nc.vector.tensor_scalar(
                    out=mg1w, in0=hh1w, scalar1=1.0, scalar2=None, op0=sub
                )
                # u1n = h1 * mg1 = -h1*g1  (TT-1024)
                u1nw = l1p.tile([128, 2 * B], F16, tag="u1nw", name="u1nw")
                nc.vector.tensor_mul(out=u1nw, in0=h1w, in1=mg1w)

                # ================= layer 2 matmuls =================
                pa2a = paP.tile([128, B], F32, tag="pa", name="pa2a")
                nc.tensor.matmul(pa2a, W2s, h1w[:, 0:B])
                pa2b = paP.tile([128, B], F32, tag="pa", name="pa2b")
                nc.tensor.matmul(pa2b, W2s, h1w[:, B : 2 * B])
                # dzt2 = (az|at) per tile; az = Z2^T mg1 = -sqrt2*a2z',
                # at = T2^T mg1 = -a2t'
                dz2a = dztP.tile([128, 2 * B], F32, tag="dzt", name="dz2a")
                nc.tensor.matmul(dz2a[:, 0:B], Z2s, mg1w[:, 0:B])
                nc.tensor.matmul(dz2a[:, B : 2 * B], T2s, mg1w[:, 0:B])
                dz2b = dztP.tile([128, 2 * B], F32, tag="dzt", name="dz2b")
                nc.tensor.matmul(dz2b[:, 0:B], Z2s, mg1w[:, B : 2 * B])
                nc.tensor.matmul(dz2b[:, B : 2 * B], T2s, mg1w[:, B : 2 * B])
                # app2 = ZZ2^T u1n = a2''  (accumulates -v2 later)
                app2a = appP.tile([128, B], F32, tag="app", name="app2a")
                nc.tensor.matmul(app2a, ZZ2s, u1nw[:, 0:B], start=True, stop=False)
                app2b = appP.tile([128, B], F32, tag="app", name="app2b")
                nc.tensor.matmul(
                    app2b, ZZ2s, u1nw[:, B : 2 * B], start=True, stop=False
                )

                # ================= layer 2 pointwise =================
                h2w = l2p.tile([128, 2 * B], F16, tag="h2w", name="h2w")
                nc.scalar.activation(out=h2w[:, 0:B], in_=pa2a, func=Tanh, bias=b2s)
                nc.scalar.activation(
                    out=h2w[:, B : 2 * B], in_=pa2b, func=Tanh, bias=b2s
                )
                hh2w = l2p.tile([128, 2 * B], F16, tag="hh2w", name="hh2w")
                nc.vector.tensor_mul(out=hh2w, in0=h2w, in1=h2w)
                # sq2 = Square(az_psum) = 2*a2z'^2
                sq2w = l2p.tile([128, 2 * B], F16, tag="sq2w", name="sq2w")
                nc.scalar.activation(out=sq2w[:, 0:B], in_=dz2a[:, 0:B], func=Square)
                nc.scalar.activation(
                    out=sq2w[:, B : 2 * B], in_=dz2b[:, 0:B], func=Square
                )
                # v2 = h2 * sq2
                v2w = l2p.tile([128, 2 * B], F16, tag="v2w", name="v2w")
                nc.gpsimd.tensor_mul(
                    out=v2w[:, 0:B], in0=h2w[:, 0:B], in1=sq2w[:, 0:B]
                )
                nc.gpsimd.tensor_mul(
                    out=v2w[:, B : 2 * B], in0=h2w[:, B : 2 * B],
                    in1=sq2w[:, B : 2 * B],
                )
                # mzt2 = (hh2-1)*(az|at): tz2 = sqrt2*h2z', tt2 = h2t'
                # halves land B*2 apart: ztw = [tz_a|tz_b|tt_a|tt_b]
                ztw = l2p.tile([128, 4 * B], F16, tag="ztw", name="ztw")
                stt(
                    nc.vector,
                    split2(
                        bass.AP(tensor=ztw.tensor, offset=ztw[:].offset,
                                ap=ztw[:].ap),
                        2 * B,
                    ),
                    bc2(hh2w[:, 0:B]),
                    dz2a[:],
                )
                stt(
                    nc.vector,
                    split2(
                        bass.AP(tensor=ztw.tensor, offset=ztw[:].offset + B,
                                ap=ztw[:].ap),
                        2 * B,
                    ),
                    bc2(hh2w[:, B : 2 * B]),
                    dz2b[:],
                )
                # i2 = app2 - v2 (PE accumulate)
                nc.tensor.matmul(app2a, negIs, v2w[:, 0:B], start=False, stop=True)
                nc.tensor.matmul(
                    app2b, negIs, v2w[:, B : 2 * B], start=False, stop=True
                )
                # mhpp2 = (hh2-1)*i2 = -h2''
                mhpp2 = l2p.tile([128, 2 * B], F16, tag="mhpp2", name="mhpp2")
                stt(nc.vector, mhpp2[:, 0:B], hh2w[:, 0:B], app2a[:])
                stt(nc.vector, mhpp2[:, B : 2 * B], hh2w[:, B : 2 * B], app2b[:])

                # ================= layer 3 matmuls (pair-packed) =========
                tz2a = ztw[:, 0:B]
                tz2b = ztw[:, B : 2 * B]
                tt2a = ztw[:, 2 * B : 3 * B]
                tt2b = ztw[:, 3 * B : 4 * B]
                pa3 = paP.tile([128, B], F32, tag="pa", name="pa3")
                nc.tensor.matmul(pa3[0:64], W3s, h2w[:, 0:B])
                nc.tensor.matmul(pa3[64:128], W3s, h2w[:, B : 2 * B])
                dz3 = dztP.tile([128, 2 * B], F32, tag="dzt", name="dz3")
                nc.tensor.matmul(dz3[0:64, 0:B], W3s, tz2a)  # sqrt2*a3z'
                nc.tensor.matmul(dz3[64:128, 0:B], W3s, tz2b)
                nc.tensor.matmul(dz3[0:64, B : 2 * B], W3s, tt2a)  # a3t'
                nc.tensor.matmul(dz3[64:128, B : 2 * B], W3s, tt2b)
                papp3 = appP.tile([128, B], F32, tag="app", name="papp3")
                nc.tensor.matmul(
                    papp3[0:64], W3ns, mhpp2[:, 0:B], start=True, stop=False
                )
                nc.tensor.matmul(
                    papp3[64:128], W3ns, mhpp2[:, B : 2 * B], start=True, stop=False
                )

                # ================= layer 3 pointwise =================
                h3t = l3p.tile([128, B], F16, tag="h3", name="h3")
                h3 = h3t[:]
                nc.scalar.activation(out=h3, in_=pa3, func=Tanh, bias=b3ds)
                sq3 = l3p.tile([128, B], F16, tag="sq3", name="sq3")
                nc.scalar.activation(out=sq3, in_=dz3[:, 0:B], func=Square)
                hh3t = l3p.tile([128, B], F16, tag="hh3", name="hh3")
                hh3 = hh3t[:]
                nc.scalar.activation(out=hh3, in_=h3, func=Square)
                v3 = l3p.tile([128, B], F16, tag="v3", name="v3")
                nc.gpsimd.tensor_mul(out=v3, in0=h3, in1=sq3)
                # mzt3 = (hh3-1)*(az3|at3) = (-sqrt2*h3z' | -h3t')
                zt3 = l3p.tile([128, 2 * B], F16, tag="zt3", name="zt3")
                stt(nc.vector, zt3[:], bc2(hh3), dz3[:])
                nc.tensor.matmul(papp3, negIs, v3, start=False, stop=True)
                # mhpp3 = (hh3-1)*i3 = -h3''
                mhpp3 = l3p.tile([128, B], F16, tag="mhpp3", name="mhpp3")
                stt(nc.vector, mhpp3, hh3, papp3[:])

                # ================= layer 4 =================
                p4 = appP.tile([8, B], F32, tag="app", name="p4")
                nc.tensor.matmul(p4, W4hs, h3, start=True, stop=False)
                nc.tensor.matmul(p4, W4zs, zt3[:, 0:B], start=False, stop=False)
                nc.tensor.matmul(p4, W4ts, zt3[:, B : 2 * B], start=False, stop=False)
                nc.tensor.matmul(p4, W4ps, mhpp3, start=False, stop=True)
                sb4 = sb4p.tile([8, B], F32, tag="sb4", name="sb4")
                nc.scalar.copy(out=sb4, in_=p4)
                ofull = out_d[:]
                o8 = bass.AP(
                    tensor=ofull.tensor,
                    offset=ofull.offset + ta * B,
                    ap=[[B, 2], [NSH, 4], [1, B]],
                )
                nc.sync.dma_start(out=o8, in_=sb4)

    nc.finalize()
    return nc


_NC_CACHE = None


def _get_nc():
    global _NC_CACHE
    if _NC_CACHE is None:
        _NC_CACHE = _build()
    return _NC_CACHE


def kernel(**inputs):
    global LAST_RESULT
    f = np.float32
    f16 = np.float16
    x = np.asarray(inputs["x"], dtype=f)
    W1 = np.asarray(inputs["W1"], dtype=f)
    b1 = np.asarray(inputs["b1"], dtype=f)
    W2 = np.asarray(inputs["W2"], dtype=f)
    b2 = np.asarray(inputs["b2"], dtype=f)
    W3 = np.asarray(inputs["W3"], dtype=f)
    b3 = np.asarray(inputs["b3"], dtype=f)
    W4 = np.asarray(inputs["W4"], dtype=f)
    b4 = np.asarray(inputs["b4"], dtype=f)

    xT = np.ascontiguousarray(x.T)  # [3, N]
    w4 = W4[:, 0].astype(f)
    SQ2 = np.sqrt(2.0).astype(f)

    W4h = np.zeros((128, 8), f)
    W4h[0:64, 0] = w4
    W4h[64:128, 4] = w4
    W4z = np.zeros((128, 8), f)
    W4z[0:64, 1] = -w4 / SQ2
    W4z[64:128, 5] = -w4 / SQ2
    W4t = np.zeros((128, 8), f)
    W4t[0:64, 2] = -w4
    W4t[64:128, 6] = -w4
    W4p = np.zeros((128, 8), f)
    W4p[0:64, 3] = -w4
    W4p[64:128, 7] = -w4

    common = {
        "W1": W1.astype(f16),
        "W2": W2.astype(f16),
        "Z2": (SQ2 * W1[0][:, None] * W2).astype(f16),
        "T2": (W1[1][:, None] * W2).astype(f16),
        "ZZ2": (2.0 * (W1[0] ** 2)[:, None] * W2).astype(f16),
        "W3": W3.astype(f16),
        "W3n": (-W3).astype(f16),
        "W4h": W4h.astype(f16),
        "W4z": W4z.astype(f16),
        "W4t": W4t.astype(f16),
        "W4p": W4p.astype(f16),
        "negI": (-np.eye(128)).astype(f16),
        "b1": np.ascontiguousarray(b1.reshape(128, 1)),
        "b2": np.ascontiguousarray(b2.reshape(128, 1)),
        "b3d": np.ascontiguousarray(np.concatenate([b3, b3]).reshape(128, 1)),
    }
    in_maps = [
        dict(
            common,
            xT=np.ascontiguousarray(xT[:, i * NSH : (i + 1) * NSH]).astype(f16),
        )
        for i in range(NCORES)
    ]

    nc = _get_nc()
    res = run_bass_kernel_spmd(nc, in_maps, list(range(NCORES)), trace=TRACE)
    LAST_RESULT = res

    full = np.concatenate(
        [res.results[i]["out"] for i in range(NCORES)], axis=1
    )  # [4, N] rows (T, Tz, Tt, Tpp)
    out = np.ascontiguousarray(full.T).astype(f)
    out[:, 0] += b4[0]
    return out
